# revision 1
# baseline (speedup 1.0000x reference)
"""Trainium2 Bass kernel for nn_Decoder (worker/task label-probability decoder).

Math:
    worker_feature = inputs[:2048, :64]          # [Wn, A]
    tau            = inputs[2048:, :16]          # [T, L]
    p1 = sigmoid(worker_feature @ W + b)         # [Wn, 1]
    p2 = (1 - p1) / (L - 1)
    P[i, j, l] = p1[i]^tau[j,l] * p2[i]^(1 - tau[j,l])
               = exp(a[i] * tau[j,l] + c[i]),  a = ln p1 - ln p2, c = ln p2

Sharding: pure data parallel over the worker axis (dim 0), 256 workers per
core across 8 cores; tau/W/b replicated. No communication.

Per-core device layout: workers on SBUF partitions (2 groups of 128), task
axis flattened on the free dimension. tau arrives striped [16, 2048] so the
load spreads over 16 DMA ports; the otherwise-idle GPSIMD engine then
replicates each stripe to all 128 partitions (partition_broadcast — an exact
fp32 copy). The scalar engine computes Exp(a*tau + c) in one pass with
per-partition scale/bias, and results stream to HBM as 2 MiB writes. The
only non-trivial HBM traffic is the 32 MiB output per core, so the kernel
runs at the DMA roofline.
"""

import numpy as np

try:
    import concourse.bass as bass  # noqa: F401
except ImportError:  # fall back to the container's repo checkout
    import sys

    for _p in ("/root/.axon_site/_ro/trn_rl_repo", "/opt/trn_rl_repo"):
        if _p not in sys.path:
            sys.path.append(_p)

import concourse.bass as bass
import concourse.tile as tile
from concourse import library_config, mybir
from concourse.bass_utils import run_bass_kernel_spmd
from concourse.vector_clock import ScopedClock

WN = 2048  # workers total
TN = 2048  # tasks
L = 16  # edge types / labels
A = 64  # ability features
NCORES = 8
WPC = WN // NCORES  # workers per core (256)
G = WPC // 128  # partition groups per core (2)
F = TN * L  # flattened task axis (32768)
CH = 2048  # tau stripe length (one partition_broadcast each)
NST = F // CH  # tau stripes (16)
OT = 4096  # free-dim elements per output tile / ACT op (2 MiB tiles)

_AF = mybir.ActivationFunctionType


class _TC(tile.TileContext):
    """TileContext legalized for a walrus that allows one sync-wait per inst.

    The walrus build in this container rejects any instruction carrying more
    than one sync-wait command. After Tile's normal scheduling + the exit
    drain/barrier, rewrite every multi-wait instruction into a chain of
    same-engine NOPs (one wait each) followed by the instruction with the
    final wait.
    """

    def _drain_and_barrier(self, tick_clock, wait_clock):
        super()._drain_and_barrier(tick_clock, wait_clock)
        self._split_multi_waits()

    def _fresh_nop(self, engine):
        inst = self.nc.engines[engine].nop(nofuse=True).ins
        self.nc.cur_bb.bb.instructions.remove(inst)
        return inst

    def _split_multi_waits(self):
        for fn in self.nc.m.functions:
            for bb in fn.blocks:
                snapshot = list(bb.instructions)
                if not any(
                    inst.sync_info and len(inst.sync_info.on_wait) > 1
                    for inst in snapshot
                ):
                    continue
                new = []
                for inst in snapshot:
                    si = inst.sync_info
                    if si is not None and si.on_wait and len(si.on_wait) > 1:
                        waits = list(si.on_wait)
                        si.on_wait = waits[-1:]
                        inst.sync_info = si
                        for wt in waits[:-1]:
                            nop = self._fresh_nop(inst.engine)
                            nop.sync_info = mybir.SyncInfo(on_wait=[wt], on_update=[])
                            new.append(nop)
                    new.append(inst)
                bb.instructions[:] = new


def build_nc():
    nc = bass.Bass("TRN2")
    wf = nc.dram_tensor("wf", [WPC, A], mybir.dt.float32, kind="ExternalInput")
    tau_in = nc.dram_tensor("tau", [NST, CH], mybir.dt.float32, kind="ExternalInput")
    tau3_in = nc.dram_tensor("tau3", [3, F], mybir.dt.bfloat16, kind="ExternalInput")
    w_in = nc.dram_tensor("W", [A], mybir.dt.float32, kind="ExternalInput")
    b_in = nc.dram_tensor("b", [1], mybir.dt.float32, kind="ExternalInput")
    out = nc.dram_tensor("out", [G, 128, F], mybir.dt.float32, kind="ExternalOutput")

    f32 = mybir.dt.float32
    bf16 = mybir.dt.bfloat16

    with _TC(nc) as tc:
        with (
            tc.tile_pool(name="const", bufs=1) as const,
            tc.tile_pool(name="reps", bufs=2) as reps,
            tc.tile_pool(name="outs", bufs=3) as outs,
            tc.tile_pool(name="psum", bufs=2, space="PSUM") as psum,
        ):
            # ---- constant / prep tiles ----
            wf_sb = const.tile([128, G, A], f32)
            nc.sync.dma_start(
                out=wf_sb, in_=wf[:].rearrange("(g p) a -> p g a", p=128)
            )

            w_ap = w_in[:]
            w_sb = const.tile([128, A], f32)
            nc.sync.dma_start(
                out=w_sb,
                in_=bass.AP(tensor=w_ap.tensor, offset=w_ap.offset, ap=[[0, 128], [1, A]]),
            )
            b_ap = b_in[:]
            b_sb = const.tile([128, 1], f32)
            nc.sync.dma_start(
                out=b_sb,
                in_=bass.AP(tensor=b_ap.tensor, offset=b_ap.offset, ap=[[0, 128], [1, 1]]),
            )

            # ---- per-worker scalars: a = ln p1 - ln p2, c = ln p2 ----
            x = const.tile([128, G], f32)
            for g in range(G):
                prod = const.tile([128, A], f32, tag=f"prod{g}")
                nc.vector.tensor_mul(prod, wf_sb[:, g, :], w_sb)
                nc.vector.reduce_sum(x[:, g : g + 1], prod, axis=mybir.AxisListType.X)

            bneg = const.tile([128, 1], f32)
            nc.vector.tensor_scalar_mul(bneg, b_sb, -1.0)
            # e = exp(-(x + b));  p1 = 1 / (1 + e)
            e = const.tile([128, G], f32)
            nc.scalar.activation(e, x, _AF.Exp, bias=bneg[:, 0:1], scale=-1.0)
            nc.vector.tensor_scalar_add(e, e, 1.0)
            p1 = const.tile([128, G], f32)
            nc.vector.reciprocal(p1, e)
            p2 = const.tile([128, G], f32)
            nc.vector.tensor_scalar(
                p2,
                p1,
                scalar1=-1.0 / (L - 1),
                scalar2=1.0 / (L - 1),
                op0=mybir.AluOpType.mult,
                op1=mybir.AluOpType.add,
            )
            lp1 = const.tile([128, G], f32)
            nc.scalar.activation(lp1, p1, _AF.Ln)
            lp2 = const.tile([128, G], f32)
            nc.scalar.activation(lp2, p2, _AF.Ln)
            a_sb = const.tile([128, G], f32)
            nc.vector.tensor_sub(a_sb, lp1, lp2)

            # ---- main loop: broadcast tau -> ACT exp -> DMA out ----
            tau_flat = tau_in[:].rearrange("s c -> (s c)")

            def emit_round(rep_ap, f0, sz, key):
                for g in range(G):
                    ot = outs.tile(
                        [128, sz], f32, tag=f"ot{g}", name=f"ot{g}_{key}", bufs=2
                    )
                    nc.scalar.activation(
                        ot,
                        rep_ap,
                        _AF.Exp,
                        bias=lp2[:, g : g + 1],
                        scale=a_sb[:, g : g + 1],
                    )
                    nc.sync.dma_start(out=out[g, :, f0 : f0 + sz], in_=ot)

            def hbm_rep(f0, sz, key):
                # Replicate straight from HBM — used only during the ramp,
                # while the HBM write stream is still idle.
                rep = reps.tile([128, sz], f32, tag="rep", name=f"rep_{key}", bufs=3)
                nc.gpsimd.dma_start(
                    out=rep,
                    in_=bass.AP(
                        tensor=tau_flat.tensor,
                        offset=tau_flat.offset + f0,
                        ap=[[0, 128], [1, sz]],
                    ),
                )
                return rep

            # bf16 3-term split of tau for the exact PE broadcast; loaded
            # in two halves on the ACT HWDGE ring (idle early, and never
            # queues behind the output writes on SP's ring).
            tau_sb = const.tile([3, F], bf16)
            nc.scalar.dma_start(out=tau_sb[:, : F // 2], in_=tau3_in[:, : F // 2])
            nc.scalar.dma_start(out=tau_sb[:, F // 2 :], in_=tau3_in[:, F // 2 :])
            ones = const.tile([3, 128], bf16)
            nc.vector.memset(ones, 1.0)

            # Round 0 in 1 MiB pieces from HBM (the write stream is idle, so
            # the 2 MiB broadcast read is free) so the first write launches
            # ASAP. Rounds 1+ use the tensor engine: ones.T @ tau_split
            # replicates tau into PSUM exactly, with zero HBM traffic.
            for h in range(OT // CH):
                rep = hbm_rep(h * CH, CH, f"w0{h}")
                emit_round(rep, h * CH, CH, f"w0{h}")

            for q in range(1, F // OT):
                ots = [
                    outs.tile([128, OT], f32, tag=f"ot{g}", name=f"ot{g}_q{q}", bufs=2)
                    for g in range(G)
                ]
                for h in range(OT // CH):
                    pt = psum.tile([128, CH], f32, tag="pt", name="pt")
                    base = q * OT + h * CH
                    for n in range(CH // 512):
                        nc.tensor.matmul(
                            pt[:, n * 512 : (n + 1) * 512],
                            ones,
                            tau_sb[:, base + n * 512 : base + (n + 1) * 512],
                            start=True,
                            stop=True,
                        )
                    for g in range(G):
                        nc.scalar.activation(
                            ots[g][:, h * CH : (h + 1) * CH],
                            pt,
                            _AF.Exp,
                            bias=lp2[:, g : g + 1],
                            scale=a_sb[:, g : g + 1],
                        )
                for g in range(G):
                    nc.sync.dma_start(
                        out=out[g, :, q * OT : (q + 1) * OT], in_=ots[g]
                    )
    return nc


def _split3_bf16(x32):
    """Exact 3-term bf16 decomposition of fp32 (hi+mid+lo == x bit-exact)."""
    import ml_dtypes

    bf = ml_dtypes.bfloat16
    hi = x32.astype(bf)
    r1 = x32 - hi.astype(np.float32)
    mid = r1.astype(bf)
    r2 = r1 - mid.astype(np.float32)
    lo = r2.astype(bf)
    return np.stack([hi, mid, lo], axis=0)


_NC = None


def kernel(inputs, W, b, worker_num=WN, task_num=TN, edge_type=L, ability_num=A, **_kw):
    global _NC
    inputs = np.ascontiguousarray(np.asarray(inputs, dtype=np.float32))
    W = np.asarray(W, dtype=np.float32).reshape(A)
    b = np.asarray(b, dtype=np.float32).reshape(1)
    assert inputs.shape == (WN + TN, A)

    wf = inputs[:WN, :A]
    tau_flat = inputs[WN:, :L].reshape(F)
    tau = np.ascontiguousarray(tau_flat.reshape(NST, CH))
    tau3 = np.ascontiguousarray(_split3_bf16(tau_flat))

    if _NC is None:
        _NC = build_nc()

    in_maps = [
        {
            "wf": np.ascontiguousarray(wf[k * WPC : (k + 1) * WPC]),
            "tau": tau,
            "tau3": tau3,
            "W": W,
            "b": b,
        }
        for k in range(NCORES)
    ]
    res = run_bass_kernel_spmd(_NC, in_maps, core_ids=list(range(NCORES)))
    parts = [r["out"].reshape(WPC, TN, L) for r in res.results]
    return np.concatenate(parts, axis=0)



# revision 4
# speedup vs baseline: 1.4649x; 1.4649x over previous
"""Trainium2 Bass kernel for nn_Decoder (worker/task label-probability decoder).

Math:
    worker_feature = inputs[:2048, :64]          # [Wn, A]
    tau            = inputs[2048:, :16]          # [T, L]
    p1 = sigmoid(worker_feature @ W + b)         # [Wn, 1]
    p2 = (1 - p1) / (L - 1)
    P[i, j, l] = p1[i]^tau[j,l] * p2[i]^(1 - tau[j,l])
               = exp(a[i] * tau[j,l] + c[i]),  a = ln p1 - ln p2, c = ln p2

Sharding: pure data parallel over the worker axis (dim 0), 256 workers per
core across 8 cores; tau/W/b replicated. No communication.

Per-core layout: workers on SBUF partitions (2 groups of 128), task axis
flattened on the free dimension. tau is shipped as an exact 3-term bf16
split; the otherwise-idle tensor engine replicates it to all 128 partitions
(ones.T @ tau3 sums the three bf16 terms in fp32 -> exact tau in PSUM).
The scalar engine computes Exp(a*tau + c) with per-partition scale/bias,
writing bf16 tiles (the 2e-2 rel-err budget dwarfs bf16's 2e-3 rounding),
which halves the HBM write traffic to 16.8 MiB/core. Output DMAs rotate
across the SP/DVE/Pool queues so per-DMA queue overheads (~2.1 us) hide
under other queues' transfers and the scalar engine never stalls: the
kernel runs at the scalar-engine exp roofline (~59 us busy), with the DMA
bus (~47 us) fully hidden beneath it.
"""

import numpy as np

try:
    import concourse.bass as bass  # noqa: F401
except ImportError:  # fall back to the container's repo checkout
    import sys

    for _p in ("/root/.axon_site/_ro/trn_rl_repo", "/opt/trn_rl_repo"):
        if _p not in sys.path:
            sys.path.append(_p)

import concourse.bass as bass
import concourse.tile as tile
from concourse import library_config, mybir
from concourse.bass_utils import run_bass_kernel_spmd
from concourse.vector_clock import ScopedClock

WN = 2048  # workers total
TN = 2048  # tasks
L = 16  # edge types / labels
A = 64  # ability features
NCORES = 8
WPC = WN // NCORES  # workers per core (256)
G = WPC // 128  # partition groups per core (2)
F = TN * L  # flattened task axis (32768)
CH = 2048  # chunk length: PSUM ping-pong granularity + per-chunk output DMA
NCH = F // CH  # chunks (16)
NPC = 4  # tau3 load pieces
PC = F // NPC  # tau3 piece length (8192)

_AF = mybir.ActivationFunctionType


class _TC(tile.TileContext):
    """TileContext legalized for a walrus that allows one sync-wait per inst.

    The walrus build in this container rejects any instruction carrying more
    than one sync-wait command. After Tile's normal scheduling + the exit
    drain/barrier, rewrite every multi-wait instruction into a chain of
    same-engine NOPs (one wait each) followed by the instruction with the
    final wait.
    """

    def _drain_and_barrier(self, tick_clock, wait_clock):
        super()._drain_and_barrier(tick_clock, wait_clock)
        self._split_multi_waits()

    def _fresh_nop(self, engine):
        inst = self.nc.engines[engine].nop(nofuse=True).ins
        self.nc.cur_bb.bb.instructions.remove(inst)
        return inst

    def _split_multi_waits(self):
        for fn in self.nc.m.functions:
            for bb in fn.blocks:
                snapshot = list(bb.instructions)
                if not any(
                    inst.sync_info and len(inst.sync_info.on_wait) > 1
                    for inst in snapshot
                ):
                    continue
                new = []
                for inst in snapshot:
                    si = inst.sync_info
                    if si is not None and si.on_wait and len(si.on_wait) > 1:
                        waits = list(si.on_wait)
                        si.on_wait = waits[-1:]
                        inst.sync_info = si
                        for wt in waits[:-1]:
                            nop = self._fresh_nop(inst.engine)
                            nop.sync_info = mybir.SyncInfo(on_wait=[wt], on_update=[])
                            new.append(nop)
                    new.append(inst)
                bb.instructions[:] = new


def build_nc():
    nc = bass.Bass("TRN2")
    wf = nc.dram_tensor("wf", [WPC, A], mybir.dt.float32, kind="ExternalInput")
    tau3_in = nc.dram_tensor("tau3", [3, F], mybir.dt.bfloat16, kind="ExternalInput")
    w_in = nc.dram_tensor("W", [A], mybir.dt.float32, kind="ExternalInput")
    b_in = nc.dram_tensor("b", [1], mybir.dt.float32, kind="ExternalInput")
    out = nc.dram_tensor("out", [G, 128, F], mybir.dt.bfloat16, kind="ExternalOutput")

    f32 = mybir.dt.float32
    bf16 = mybir.dt.bfloat16

    with _TC(nc) as tc:
        with (
            tc.tile_pool(name="const", bufs=1) as const,
            tc.tile_pool(name="outs", bufs=4) as outs,
            tc.tile_pool(name="psum", bufs=2, space="PSUM") as psum,
        ):
            # ---- tau3 (exact 3-term bf16 split of tau), loaded in pieces so
            # chunk 0's matmul only waits for piece 0. Piece 0 rides the idle
            # ACT ring; the rest go on Pool's SWDGE ring (its first output DMA
            # comes late enough).
            tau_sb = [const.tile([3, PC], bf16, name=f"tau_sb{p}") for p in range(NPC)]
            nc.scalar.dma_start(out=tau_sb[0], in_=tau3_in[:, :PC])
            for p in range(1, NPC):
                nc.gpsimd.dma_start(
                    out=tau_sb[p], in_=tau3_in[:, p * PC : (p + 1) * PC]
                )
            ones = const.tile([3, 128], bf16)
            nc.vector.memset(ones, 1.0)

            # ---- constant / prep tiles ----
            wf_sb = const.tile([128, G, A], f32)
            nc.sync.dma_start(
                out=wf_sb, in_=wf[:].rearrange("(g p) a -> p g a", p=128)
            )

            w_ap = w_in[:]
            w_sb = const.tile([128, A], f32)
            nc.scalar.dma_start(
                out=w_sb,
                in_=bass.AP(tensor=w_ap.tensor, offset=w_ap.offset, ap=[[0, 128], [1, A]]),
            )
            b_ap = b_in[:]
            b_sb = const.tile([128, 1], f32)
            nc.scalar.dma_start(
                out=b_sb,
                in_=bass.AP(tensor=b_ap.tensor, offset=b_ap.offset, ap=[[0, 128], [1, 1]]),
            )

            # ---- per-worker scalars: a = ln p1 - ln p2, c = ln p2 ----
            x = const.tile([128, G], f32)
            for g in range(G):
                prod = const.tile([128, A], f32, tag=f"prod{g}")
                nc.vector.tensor_mul(prod, wf_sb[:, g, :], w_sb)
                nc.vector.reduce_sum(x[:, g : g + 1], prod, axis=mybir.AxisListType.X)

            bneg = const.tile([128, 1], f32)
            nc.vector.tensor_scalar_mul(bneg, b_sb, -1.0)
            # e = exp(-(x + b));  p1 = 1 / (1 + e)
            e = const.tile([128, G], f32)
            nc.scalar.activation(e, x, _AF.Exp, bias=bneg[:, 0:1], scale=-1.0)
            nc.vector.tensor_scalar_add(e, e, 1.0)
            p1 = const.tile([128, G], f32)
            nc.vector.reciprocal(p1, e)
            p2 = const.tile([128, G], f32)
            nc.vector.tensor_scalar(
                p2,
                p1,
                scalar1=-1.0 / (L - 1),
                scalar2=1.0 / (L - 1),
                op0=mybir.AluOpType.mult,
                op1=mybir.AluOpType.add,
            )
            lp1 = const.tile([128, G], f32)
            nc.scalar.activation(lp1, p1, _AF.Ln)
            lp2 = const.tile([128, G], f32)
            nc.scalar.activation(lp2, p2, _AF.Ln)
            a_sb = const.tile([128, G], f32)
            nc.vector.tensor_sub(a_sb, lp1, lp2)

            # ---- main loop: PE-broadcast tau chunk -> ACT exp -> DMA out ----
            # Output DMAs rotate over the three engine rings that are idle in
            # steady state; the scalar engine only ever computes.
            dma_engines = [nc.sync, nc.gpsimd, nc.scalar]
            for c in range(NCH):
                pt = psum.tile([128, CH], f32, tag="pt", name="pt")
                base = c * CH
                piece, off = base // PC, base % PC
                for n in range(CH // 512):
                    nc.tensor.matmul(
                        pt[:, n * 512 : (n + 1) * 512],
                        ones,
                        tau_sb[piece][:, off + n * 512 : off + (n + 1) * 512],
                        start=True,
                        stop=True,
                    )
                for g in range(G):
                    ot = outs.tile(
                        [128, CH], bf16, tag=f"ot{g}", name=f"ot{g}_c{c}", bufs=4
                    )
                    nc.scalar.activation(
                        ot,
                        pt,
                        _AF.Exp,
                        bias=lp2[:, g : g + 1],
                        scale=a_sb[:, g : g + 1],
                    )
                    eng = dma_engines[(c * G + g) % len(dma_engines)]
                    eng.dma_start(out=out[g, :, base : base + CH], in_=ot)
    return nc


def _split3_bf16(x32):
    """Exact 3-term bf16 decomposition of fp32 (hi+mid+lo == x bit-exact)."""
    import ml_dtypes

    bf = ml_dtypes.bfloat16
    hi = x32.astype(bf)
    r1 = x32 - hi.astype(np.float32)
    mid = r1.astype(bf)
    r2 = r1 - mid.astype(np.float32)
    lo = r2.astype(bf)
    return np.stack([hi, mid, lo], axis=0)


_NC = None


def kernel(inputs, W, b, worker_num=WN, task_num=TN, edge_type=L, ability_num=A, **_kw):
    global _NC
    inputs = np.ascontiguousarray(np.asarray(inputs, dtype=np.float32))
    W = np.asarray(W, dtype=np.float32).reshape(A)
    b = np.asarray(b, dtype=np.float32).reshape(1)
    assert inputs.shape == (WN + TN, A)

    wf = inputs[:WN, :A]
    tau_flat = inputs[WN:, :L].reshape(F)
    tau3 = np.ascontiguousarray(_split3_bf16(tau_flat))

    if _NC is None:
        _NC = build_nc()

    in_maps = [
        {
            "wf": np.ascontiguousarray(wf[k * WPC : (k + 1) * WPC]),
            "tau3": tau3,
            "W": W,
            "b": b,
        }
        for k in range(NCORES)
    ]
    res = run_bass_kernel_spmd(_NC, in_maps, core_ids=list(range(NCORES)))
    parts = [
        r["out"].astype(np.float32).reshape(WPC, TN, L) for r in res.results
    ]
    return np.concatenate(parts, axis=0)


# revision 10
# speedup vs baseline: 1.8246x; 1.2455x over previous
"""Trainium2 Bass kernel for nn_Decoder (worker/task label-probability decoder).

Math:
    worker_feature = inputs[:2048, :64]          # [Wn, A]
    tau            = inputs[2048:, :16]          # [T, L]
    p1 = sigmoid(worker_feature @ W + b)         # [Wn, 1]
    p2 = (1 - p1) / (L - 1)
    P[i, j, l] = p1[i]^tau[j,l] * p2[i]^(1 - tau[j,l])
               = exp(a[i] * tau[j,l] + c[i]),  a = ln p1 - ln p2, c = ln p2

Sharding: pure data parallel over the worker axis (dim 0), 256 workers per
core across 8 cores; tau/W/b replicated. No communication.

Per-core layout: workers on SBUF partitions (2 groups of 128), task axis
flattened on the free dimension. tau ships as an exact 3-term bf16 split,
striped [48 x 2048] so the load is one cheap DMA; the otherwise-idle tensor
engine replicates each stripe to all 128 partitions (ones.T @ tau3 sums the
three bf16 terms in fp32 -> exact tau in PSUM). The scalar engine computes
Exp(a*tau + c) with per-partition scale/bias, writing bf16 tiles (the 2e-2
rel-err budget dwarfs bf16's 2e-3 rounding), halving HBM write traffic.

The scalar engine is the roofline (~61 us of Exp); everything else is
arranged to keep it saturated: output DMAs go only on the SP and Pool
queues, the Exp activation table is prefetched via a dummy op during the
input loads, dummy matmuls warm the PE p-state so chunk 0 isn't slowed by
the cold tensor engine, and the first/last chunks are subdivided to shrink
pipeline fill/drain.
"""

import numpy as np

try:
    import concourse.bass as bass  # noqa: F401
except ImportError:  # fall back to the container's repo checkout
    import sys

    for _p in ("/root/.axon_site/_ro/trn_rl_repo", "/opt/trn_rl_repo"):
        if _p not in sys.path:
            sys.path.append(_p)

import concourse.bass as bass
import concourse.tile as tile
from concourse import library_config, mybir
from concourse.bass_utils import run_bass_kernel_spmd
from concourse.vector_clock import ScopedClock

WN = 2048  # workers total
TN = 2048  # tasks
L = 16  # edge types / labels
A = 64  # ability features
NCORES = 8
WPC = WN // NCORES  # workers per core (256)
G = WPC // 128  # partition groups per core (2)
F = TN * L  # flattened task axis (32768)
CH = 2048  # stripe length: PSUM ping-pong granularity + per-chunk output DMA
NST = F // CH  # tau stripes (16)
NWARM = 16  # dummy matmuls to warm the PE p-state

# Work queue: (stripe, free-offset, size). First/last stripes are subdivided
# so the first ACT fires as early as possible and the drain tail is short.
PIECES = (
    [(0, 0, 512), (0, 512, 512), (0, 1024, 1024)]
    + [(s, 0, CH) for s in range(1, NST - 1)]
    + [(NST - 1, 0, 1024), (NST - 1, 1024, 512), (NST - 1, 1536, 512)]
)

_AF = mybir.ActivationFunctionType


class _TC(tile.TileContext):
    """TileContext legalized for a walrus that allows one sync-wait per inst.

    The walrus build in this container rejects any instruction carrying more
    than one sync-wait command. After Tile's normal scheduling + the exit
    drain/barrier, rewrite every multi-wait instruction into a chain of
    same-engine NOPs (one wait each) followed by the instruction with the
    final wait.
    """

    def _drain_and_barrier(self, tick_clock, wait_clock):
        super()._drain_and_barrier(tick_clock, wait_clock)
        self._split_multi_waits()

    def _fresh_nop(self, engine):
        inst = self.nc.engines[engine].nop(nofuse=True).ins
        self.nc.cur_bb.bb.instructions.remove(inst)
        return inst

    def _split_multi_waits(self):
        for fn in self.nc.m.functions:
            for bb in fn.blocks:
                snapshot = list(bb.instructions)
                if not any(
                    inst.sync_info and len(inst.sync_info.on_wait) > 1
                    for inst in snapshot
                ):
                    continue
                new = []
                for inst in snapshot:
                    si = inst.sync_info
                    if si is not None and si.on_wait and len(si.on_wait) > 1:
                        waits = list(si.on_wait)
                        si.on_wait = waits[-1:]
                        inst.sync_info = si
                        for wt in waits[:-1]:
                            nop = self._fresh_nop(inst.engine)
                            nop.sync_info = mybir.SyncInfo(on_wait=[wt], on_update=[])
                            new.append(nop)
                    new.append(inst)
                bb.instructions[:] = new


def build_nc():
    nc = bass.Bass("TRN2")
    wf = nc.dram_tensor("wf", [WPC, A], mybir.dt.float32, kind="ExternalInput")
    # tau3[3*s + t] = bf16 term t of tau stripe s (exact hi/mid/lo split).
    tau3_in = nc.dram_tensor(
        "tau3", [3 * NST, CH], mybir.dt.bfloat16, kind="ExternalInput"
    )
    # mask[k, 128*s + i] = 1.0 iff row k belongs to stripe s: used as matmul
    # weights so ones.T-style contraction over all 48 rows extracts exactly
    # stripe s's three bf16 terms (their fp32 sum = tau, bit-exact).
    mask_in = nc.dram_tensor(
        "mask", [3 * NST, 128 * NST], mybir.dt.bfloat16, kind="ExternalInput"
    )
    w_in = nc.dram_tensor("W", [A], mybir.dt.float32, kind="ExternalInput")
    b_in = nc.dram_tensor("b", [1], mybir.dt.float32, kind="ExternalInput")
    out = nc.dram_tensor("out", [G, 128, F], mybir.dt.bfloat16, kind="ExternalOutput")

    f32 = mybir.dt.float32
    bf16 = mybir.dt.bfloat16

    with _TC(nc) as tc:
        with (
            tc.tile_pool(name="const", bufs=1) as const,
            tc.tile_pool(name="outs", bufs=3) as outs,
            tc.tile_pool(name="psum", bufs=2, space="PSUM") as psum,
        ):
            ones = const.tile([3, 128], bf16)
            nc.vector.memset(ones, 1.0)

            # Prefetch the Exp/Ln activation table while inputs load: the
            # first table load costs 1.28 us and must not sit on the
            # prologue's critical path.
            scr_act = const.tile([3, 128], bf16)
            nc.scalar.activation(scr_act, ones, _AF.Exp)

            # Warm the PE p-state: dummy matmuls keep the tensor engine
            # continuously busy from t~0 so the real chunk-0 matmuls run at
            # full clock instead of the 0.65 GHz cold state.
            for wi in range(NWARM):
                wpt = psum.tile([128, CH], f32, tag="pt", name=f"warm{wi}")
                nc.tensor.matmul(
                    wpt[:, 0:128], ones, ones, start=True, stop=True
                )

            # ---- input loads: wf/W/b on SP, tau3 on Pool (one 4KB/partition
            # DMA); the ACT ring never carries a DMA.
            wf_sb = const.tile([128, G, A], f32)
            nc.sync.dma_start(
                out=wf_sb, in_=wf[:].rearrange("(g p) a -> p g a", p=128)
            )
            w_ap = w_in[:]
            w_sb = const.tile([128, A], f32)
            nc.sync.dma_start(
                out=w_sb,
                in_=bass.AP(tensor=w_ap.tensor, offset=w_ap.offset, ap=[[0, 128], [1, A]]),
            )
            b_ap = b_in[:]
            b_sb = const.tile([128, 1], f32)
            nc.sync.dma_start(
                out=b_sb,
                in_=bass.AP(tensor=b_ap.tensor, offset=b_ap.offset, ap=[[0, 128], [1, 1]]),
            )
            tau_sb = const.tile([3 * NST, CH], bf16)
            nc.gpsimd.dma_start(out=tau_sb, in_=tau3_in[:])
            mask_sb = const.tile([3 * NST, 128 * NST], bf16)
            nc.gpsimd.dma_start(out=mask_sb, in_=mask_in[:])

            # ---- per-worker scalars: a = ln p1 - ln p2, c = ln p2 ----
            x = const.tile([128, G], f32)
            for g in range(G):
                prod = const.tile([128, A], f32, tag=f"prod{g}")
                nc.vector.tensor_mul(prod, wf_sb[:, g, :], w_sb)
                nc.vector.reduce_sum(x[:, g : g + 1], prod, axis=mybir.AxisListType.X)

            bneg = const.tile([128, 1], f32)
            nc.vector.tensor_scalar_mul(bneg, b_sb, -1.0)
            # e = exp(-(x + b));  p1 = 1 / (1 + e)
            e = const.tile([128, G], f32)
            nc.scalar.activation(e, x, _AF.Exp, bias=bneg[:, 0:1], scale=-1.0)
            nc.vector.tensor_scalar_add(e, e, 1.0)
            p1 = const.tile([128, G], f32)
            nc.vector.reciprocal(p1, e)
            p2 = const.tile([128, G], f32)
            nc.vector.tensor_scalar(
                p2,
                p1,
                scalar1=-1.0 / (L - 1),
                scalar2=1.0 / (L - 1),
                op0=mybir.AluOpType.mult,
                op1=mybir.AluOpType.add,
            )
            lp1 = const.tile([128, G], f32)
            nc.scalar.activation(lp1, p1, _AF.Ln)
            lp2 = const.tile([128, G], f32)
            nc.scalar.activation(lp2, p2, _AF.Ln)
            a_sb = const.tile([128, G], f32)
            nc.vector.tensor_sub(a_sb, lp1, lp2)

            # ---- main loop: PE-broadcast tau piece -> ACT exp -> DMA out ----
            dma_engines = [nc.sync, nc.gpsimd]
            qi = 0
            for pi, (s, f0, sz) in enumerate(PIECES):
                pt = psum.tile([128, CH], f32, tag="pt", name=f"pt{pi}")
                for n in range(0, sz, 512):
                    nc.tensor.matmul(
                        pt[:, f0 + n : f0 + n + min(512, sz - n)],
                        mask_sb[:, 128 * s : 128 * (s + 1)],
                        tau_sb[:, f0 + n : f0 + n + min(512, sz - n)],
                        start=True,
                        stop=True,
                    )
                base = s * CH + f0
                for g in range(G):
                    ot = outs.tile(
                        [128, CH], bf16, tag=f"ot{g}", name=f"ot{g}_p{pi}", bufs=3
                    )
                    nc.scalar.activation(
                        ot[:, :sz],
                        pt[:, f0 : f0 + sz],
                        _AF.Exp,
                        bias=lp2[:, g : g + 1],
                        scale=a_sb[:, g : g + 1],
                    )
                    dma_engines[qi % 2].dma_start(
                        out=out[g, :, base : base + sz], in_=ot[:, :sz]
                    )
                    qi += 1
    return nc


def _split3_bf16(x32):
    """Exact 3-term bf16 decomposition of fp32 (hi+mid+lo == x bit-exact)."""
    import ml_dtypes

    bf = ml_dtypes.bfloat16
    hi = x32.astype(bf)
    r1 = x32 - hi.astype(np.float32)
    mid = r1.astype(bf)
    r2 = r1 - mid.astype(np.float32)
    lo = r2.astype(bf)
    return np.stack([hi, mid, lo], axis=0)


def _pack_tau3(tau_flat):
    """[3, F] split -> [48, 2048]: row 3*s + t = term t of stripe s."""
    t3 = _split3_bf16(tau_flat)  # [3, F]
    return np.ascontiguousarray(
        t3.reshape(3, NST, CH).transpose(1, 0, 2).reshape(3 * NST, CH)
    )


def _stripe_mask():
    """[48, 16*128] bf16: mask[k, 128*s + i] = 1.0 iff k // 3 == s."""
    import ml_dtypes

    m = np.zeros((3 * NST, NST, 128), dtype=ml_dtypes.bfloat16)
    for s in range(NST):
        m[3 * s : 3 * (s + 1), s, :] = 1.0
    return np.ascontiguousarray(m.reshape(3 * NST, NST * 128))


_NC = None


def kernel(inputs, W, b, worker_num=WN, task_num=TN, edge_type=L, ability_num=A, **_kw):
    global _NC
    inputs = np.ascontiguousarray(np.asarray(inputs, dtype=np.float32))
    W = np.asarray(W, dtype=np.float32).reshape(A)
    b = np.asarray(b, dtype=np.float32).reshape(1)
    assert inputs.shape == (WN + TN, A)

    wf = inputs[:WN, :A]
    tau3 = _pack_tau3(inputs[WN:, :L].reshape(F))

    if _NC is None:
        _NC = build_nc()

    mask = _stripe_mask()
    in_maps = [
        {
            "wf": np.ascontiguousarray(wf[k * WPC : (k + 1) * WPC]),
            "tau3": tau3,
            "mask": mask,
            "W": W,
            "b": b,
        }
        for k in range(NCORES)
    ]
    res = run_bass_kernel_spmd(_NC, in_maps, core_ids=list(range(NCORES)))
    parts = [
        r["out"].astype(np.float32).reshape(WPC, TN, L) for r in res.results
    ]
    return np.concatenate(parts, axis=0)


# revision 13
# speedup vs baseline: 1.8588x; 1.0188x over previous
"""Trainium2 Bass kernel for nn_Decoder (worker/task label-probability decoder).

Math:
    worker_feature = inputs[:2048, :64]          # [Wn, A]
    tau            = inputs[2048:, :16]          # [T, L]
    p1 = sigmoid(worker_feature @ W + b)         # [Wn, 1]
    p2 = (1 - p1) / (L - 1)
    P[i, j, l] = p1[i]^tau[j,l] * p2[i]^(1 - tau[j,l])
               = exp(a[i] * tau[j,l] + c[i]),  a = ln p1 - ln p2, c = ln p2

Sharding: pure data parallel over the worker axis (dim 0), 256 workers per
core across 8 cores; tau/W/b replicated. No communication.

Per-core layout: workers on SBUF partitions (2 groups of 128), task axis
flattened on the free dimension. tau ships as an exact 3-term bf16 split,
striped [48 x 2048] so the load is one cheap DMA; the otherwise-idle tensor
engine replicates each stripe to all 128 partitions (ones.T @ tau3 sums the
three bf16 terms in fp32 -> exact tau in PSUM). The scalar engine computes
Exp(a*tau + c) with per-partition scale/bias, writing bf16 tiles (the 2e-2
rel-err budget dwarfs bf16's 2e-3 rounding), halving HBM write traffic.

The scalar engine is the roofline (~61 us of Exp); everything else is
arranged to keep it saturated: output DMAs go only on the SP and Pool
queues, the Exp activation table is prefetched via a dummy op during the
input loads, dummy matmuls warm the PE p-state so chunk 0 isn't slowed by
the cold tensor engine, and the first/last chunks are subdivided to shrink
pipeline fill/drain.
"""

import numpy as np

try:
    import concourse.bass as bass  # noqa: F401
except ImportError:  # fall back to the container's repo checkout
    import sys

    for _p in ("/root/.axon_site/_ro/trn_rl_repo", "/opt/trn_rl_repo"):
        if _p not in sys.path:
            sys.path.append(_p)

import concourse.bass as bass
import concourse.tile as tile
from concourse import library_config, mybir
from concourse.bass_utils import run_bass_kernel_spmd
from concourse.vector_clock import ScopedClock

WN = 2048  # workers total
TN = 2048  # tasks
L = 16  # edge types / labels
A = 64  # ability features
NCORES = 8
WPC = WN // NCORES  # workers per core (256)
G = WPC // 128  # partition groups per core (2)
F = TN * L  # flattened task axis (32768)
CH = 2048  # stripe length: PSUM ping-pong granularity + per-chunk output DMA
NST = F // CH  # tau stripes (16)
NWARM = 24  # dummy matmuls to warm the PE p-state

# Work queue: (stripe, free-offset, size). First/last stripes are subdivided
# so the first ACT fires as early as possible and the drain tail is short.
PIECES = (
    [(0, 0, 512), (0, 512, 512), (0, 1024, 1024)]
    + [(s, 0, CH) for s in range(1, NST - 1)]
    + [(NST - 1, 0, 1024), (NST - 1, 1024, 512), (NST - 1, 1536, 512)]
)

_AF = mybir.ActivationFunctionType


class _TC(tile.TileContext):
    """TileContext legalized for a walrus that allows one sync-wait per inst.

    The walrus build in this container rejects any instruction carrying more
    than one sync-wait command. After Tile's normal scheduling + the exit
    drain/barrier, rewrite every multi-wait instruction into a chain of
    same-engine NOPs (one wait each) followed by the instruction with the
    final wait.
    """

    def _drain_and_barrier(self, tick_clock, wait_clock):
        super()._drain_and_barrier(tick_clock, wait_clock)
        self._split_multi_waits()

    def _fresh_nop(self, engine):
        inst = self.nc.engines[engine].nop(nofuse=True).ins
        self.nc.cur_bb.bb.instructions.remove(inst)
        return inst

    def _split_multi_waits(self):
        for fn in self.nc.m.functions:
            for bb in fn.blocks:
                snapshot = list(bb.instructions)
                if not any(
                    inst.sync_info and len(inst.sync_info.on_wait) > 1
                    for inst in snapshot
                ):
                    continue
                new = []
                for inst in snapshot:
                    si = inst.sync_info
                    if si is not None and si.on_wait and len(si.on_wait) > 1:
                        waits = list(si.on_wait)
                        si.on_wait = waits[-1:]
                        inst.sync_info = si
                        for wt in waits[:-1]:
                            nop = self._fresh_nop(inst.engine)
                            nop.sync_info = mybir.SyncInfo(on_wait=[wt], on_update=[])
                            new.append(nop)
                    new.append(inst)
                bb.instructions[:] = new


def build_nc():
    nc = bass.Bass("TRN2")
    wf = nc.dram_tensor("wf", [WPC, A], mybir.dt.float32, kind="ExternalInput")
    # tau3[3*s + t] = bf16 term t of tau stripe s (exact hi/mid/lo split).
    tau3_in = nc.dram_tensor(
        "tau3", [3 * NST, CH], mybir.dt.bfloat16, kind="ExternalInput"
    )
    # mask[k, 128*s + i] = 1.0 iff row k belongs to stripe s: used as matmul
    # weights so ones.T-style contraction over all 48 rows extracts exactly
    # stripe s's three bf16 terms (their fp32 sum = tau, bit-exact).
    mask_in = nc.dram_tensor(
        "mask", [3 * NST, 128 * NST], mybir.dt.bfloat16, kind="ExternalInput"
    )
    w_in = nc.dram_tensor("W", [A], mybir.dt.float32, kind="ExternalInput")
    b_in = nc.dram_tensor("b", [1], mybir.dt.float32, kind="ExternalInput")
    out = nc.dram_tensor("out", [G, 128, F], mybir.dt.bfloat16, kind="ExternalOutput")

    f32 = mybir.dt.float32
    bf16 = mybir.dt.bfloat16

    with _TC(nc) as tc:
        with (
            tc.tile_pool(name="const", bufs=1) as const,
            tc.tile_pool(name="outs", bufs=3) as outs,
            tc.tile_pool(name="psum", bufs=2, space="PSUM") as psum,
        ):
            ones = const.tile([3, 128], bf16)
            nc.vector.memset(ones, 1.0)

            # Prefetch the Exp/Ln activation table while inputs load: the
            # first table load costs 1.28 us and must not sit on the
            # prologue's critical path.
            scr_act = const.tile([3, 128], bf16)
            nc.scalar.activation(scr_act, ones, _AF.Exp)

            # Warm the PE p-state: dummy matmuls keep the tensor engine
            # continuously busy from t~0 so the real chunk-0 matmuls run at
            # full clock instead of the 0.65 GHz cold state.
            for wi in range(NWARM):
                wpt = psum.tile([128, CH], f32, tag="pt", name=f"warm{wi}")
                nc.tensor.matmul(
                    wpt[:, 0:128], ones, ones, start=True, stop=True
                )

            # ---- input loads: wf/W/b on SP, tau3 on Pool (one 4KB/partition
            # DMA); the ACT ring never carries a DMA.
            wf_sb = const.tile([128, G, A], f32)
            nc.sync.dma_start(
                out=wf_sb, in_=wf[:].rearrange("(g p) a -> p g a", p=128)
            )
            w_ap = w_in[:]
            w_sb = const.tile([128, A], f32)
            nc.sync.dma_start(
                out=w_sb,
                in_=bass.AP(tensor=w_ap.tensor, offset=w_ap.offset, ap=[[0, 128], [1, A]]),
            )
            b_ap = b_in[:]
            b_sb = const.tile([128, 1], f32)
            nc.sync.dma_start(
                out=b_sb,
                in_=bass.AP(tensor=b_ap.tensor, offset=b_ap.offset, ap=[[0, 128], [1, 1]]),
            )
            # tau/mask head pieces cover exactly what chunk 0's first matmul
            # reads, so the PE can start ~2.5us before the full tiles land.
            tau_sb = const.tile([3 * NST, CH], bf16)
            mask_sb = const.tile([3 * NST, 128 * NST], bf16)
            nc.gpsimd.dma_start(out=tau_sb[:, :512], in_=tau3_in[:, :512])
            nc.gpsimd.dma_start(out=mask_sb[:, :128], in_=mask_in[:, :128])
            nc.gpsimd.dma_start(out=tau_sb[:, 512:], in_=tau3_in[:, 512:])
            nc.gpsimd.dma_start(out=mask_sb[:, 128:], in_=mask_in[:, 128:])

            # ---- per-worker scalars: a = ln p1 - ln p2, c = ln p2 ----
            x = const.tile([128, G], f32)
            for g in range(G):
                prod = const.tile([128, A], f32, tag=f"prod{g}")
                nc.vector.tensor_mul(prod, wf_sb[:, g, :], w_sb)
                nc.vector.reduce_sum(x[:, g : g + 1], prod, axis=mybir.AxisListType.X)

            bneg = const.tile([128, 1], f32)
            nc.vector.tensor_scalar_mul(bneg, b_sb, -1.0)
            # e = exp(-(x + b));  p1 = 1 / (1 + e)
            e = const.tile([128, G], f32)
            nc.scalar.activation(e, x, _AF.Exp, bias=bneg[:, 0:1], scale=-1.0)
            nc.vector.tensor_scalar_add(e, e, 1.0)
            p1 = const.tile([128, G], f32)
            nc.vector.reciprocal(p1, e)
            p2 = const.tile([128, G], f32)
            nc.vector.tensor_scalar(
                p2,
                p1,
                scalar1=-1.0 / (L - 1),
                scalar2=1.0 / (L - 1),
                op0=mybir.AluOpType.mult,
                op1=mybir.AluOpType.add,
            )
            lp1 = const.tile([128, G], f32)
            nc.scalar.activation(lp1, p1, _AF.Ln)
            lp2 = const.tile([128, G], f32)
            nc.scalar.activation(lp2, p2, _AF.Ln)
            a_sb = const.tile([128, G], f32)
            nc.vector.tensor_sub(a_sb, lp1, lp2)

            # ---- main loop: PE-broadcast tau piece -> ACT exp -> DMA out ----
            dma_engines = [nc.sync, nc.gpsimd]
            qi = 0
            for pi, (s, f0, sz) in enumerate(PIECES):
                pt = psum.tile([128, CH], f32, tag="pt", name=f"pt{pi}")
                for n in range(0, sz, 512):
                    nc.tensor.matmul(
                        pt[:, f0 + n : f0 + n + min(512, sz - n)],
                        mask_sb[:, 128 * s : 128 * (s + 1)],
                        tau_sb[:, f0 + n : f0 + n + min(512, sz - n)],
                        start=True,
                        stop=True,
                    )
                base = s * CH + f0
                for g in range(G):
                    ot = outs.tile(
                        [128, CH], bf16, tag=f"ot{g}", name=f"ot{g}_p{pi}", bufs=4
                    )
                    nc.scalar.activation(
                        ot[:, :sz],
                        pt[:, f0 : f0 + sz],
                        _AF.Exp,
                        bias=lp2[:, g : g + 1],
                        scale=a_sb[:, g : g + 1],
                    )
                    dma_engines[qi % 2].dma_start(
                        out=out[g, :, base : base + sz], in_=ot[:, :sz]
                    )
                    qi += 1
    return nc


def _split3_bf16(x32):
    """Exact 3-term bf16 decomposition of fp32 (hi+mid+lo == x bit-exact)."""
    import ml_dtypes

    bf = ml_dtypes.bfloat16
    hi = x32.astype(bf)
    r1 = x32 - hi.astype(np.float32)
    mid = r1.astype(bf)
    r2 = r1 - mid.astype(np.float32)
    lo = r2.astype(bf)
    return np.stack([hi, mid, lo], axis=0)


def _pack_tau3(tau_flat):
    """[3, F] split -> [48, 2048]: row 3*s + t = term t of stripe s."""
    t3 = _split3_bf16(tau_flat)  # [3, F]
    return np.ascontiguousarray(
        t3.reshape(3, NST, CH).transpose(1, 0, 2).reshape(3 * NST, CH)
    )


def _stripe_mask():
    """[48, 16*128] bf16: mask[k, 128*s + i] = 1.0 iff k // 3 == s."""
    import ml_dtypes

    m = np.zeros((3 * NST, NST, 128), dtype=ml_dtypes.bfloat16)
    for s in range(NST):
        m[3 * s : 3 * (s + 1), s, :] = 1.0
    return np.ascontiguousarray(m.reshape(3 * NST, NST * 128))


_NC = None


def kernel(inputs, W, b, worker_num=WN, task_num=TN, edge_type=L, ability_num=A, **_kw):
    global _NC
    inputs = np.ascontiguousarray(np.asarray(inputs, dtype=np.float32))
    W = np.asarray(W, dtype=np.float32).reshape(A)
    b = np.asarray(b, dtype=np.float32).reshape(1)
    assert inputs.shape == (WN + TN, A)

    wf = inputs[:WN, :A]
    tau3 = _pack_tau3(inputs[WN:, :L].reshape(F))

    if _NC is None:
        _NC = build_nc()

    mask = _stripe_mask()
    in_maps = [
        {
            "wf": np.ascontiguousarray(wf[k * WPC : (k + 1) * WPC]),
            "tau3": tau3,
            "mask": mask,
            "W": W,
            "b": b,
        }
        for k in range(NCORES)
    ]
    res = run_bass_kernel_spmd(_NC, in_maps, core_ids=list(range(NCORES)))
    parts = [
        r["out"].astype(np.float32).reshape(WPC, TN, L) for r in res.results
    ]
    return np.concatenate(parts, axis=0)


# revision 14
# speedup vs baseline: 1.8590x; 1.0001x over previous
"""Trainium2 Bass kernel for nn_Decoder (worker/task label-probability decoder).

Math:
    worker_feature = inputs[:2048, :64]          # [Wn, A]
    tau            = inputs[2048:, :16]          # [T, L]
    p1 = sigmoid(worker_feature @ W + b)         # [Wn, 1]
    p2 = (1 - p1) / (L - 1)
    P[i, j, l] = p1[i]^tau[j,l] * p2[i]^(1 - tau[j,l])
               = exp(a[i] * tau[j,l] + c[i]),  a = ln p1 - ln p2, c = ln p2

Sharding: pure data parallel over the worker axis (dim 0), 256 workers per
core across 8 cores; tau/W/b replicated. No communication.

Per-core layout: workers on SBUF partitions (2 groups of 128), task axis
flattened on the free dimension. tau ships as an exact 3-term bf16 split,
striped [48 x 2048] so the load is one cheap DMA; the otherwise-idle tensor
engine replicates each stripe to all 128 partitions (ones.T @ tau3 sums the
three bf16 terms in fp32 -> exact tau in PSUM). The scalar engine computes
Exp(a*tau + c) with per-partition scale/bias, writing bf16 tiles (the 2e-2
rel-err budget dwarfs bf16's 2e-3 rounding), halving HBM write traffic.

The scalar engine is the roofline (~61 us of Exp); everything else is
arranged to keep it saturated: output DMAs go only on the SP and Pool
queues, the Exp activation table is prefetched via a dummy op during the
input loads, dummy matmuls warm the PE p-state so chunk 0 isn't slowed by
the cold tensor engine, and the first/last chunks are subdivided to shrink
pipeline fill/drain.
"""

import numpy as np

try:
    import concourse.bass as bass  # noqa: F401
except ImportError:  # fall back to the container's repo checkout
    import sys

    for _p in ("/root/.axon_site/_ro/trn_rl_repo", "/opt/trn_rl_repo"):
        if _p not in sys.path:
            sys.path.append(_p)

import concourse.bass as bass
import concourse.tile as tile
from concourse import library_config, mybir
from concourse.bass_utils import run_bass_kernel_spmd
from concourse.vector_clock import ScopedClock

WN = 2048  # workers total
TN = 2048  # tasks
L = 16  # edge types / labels
A = 64  # ability features
NCORES = 8
WPC = WN // NCORES  # workers per core (256)
G = WPC // 128  # partition groups per core (2)
F = TN * L  # flattened task axis (32768)
CH = 2048  # stripe length: PSUM ping-pong granularity + per-chunk output DMA
NST = F // CH  # tau stripes (16)
NWARM = 24  # dummy matmuls to warm the PE p-state

# Work queue: (stripe, free-offset, size). First/last stripes are subdivided
# so the first ACT fires as early as possible and the drain tail is short.
PIECES = (
    [(0, 0, 512), (0, 512, 512), (0, 1024, 1024)]
    + [(s, 0, CH) for s in range(1, NST - 1)]
    + [(NST - 1, 0, 1024), (NST - 1, 1024, 512), (NST - 1, 1536, 512)]
)

_AF = mybir.ActivationFunctionType


class _TC(tile.TileContext):
    """TileContext legalized for a walrus that allows one sync-wait per inst.

    The walrus build in this container rejects any instruction carrying more
    than one sync-wait command. After Tile's normal scheduling + the exit
    drain/barrier, rewrite every multi-wait instruction into a chain of
    same-engine NOPs (one wait each) followed by the instruction with the
    final wait.
    """

    def _drain_and_barrier(self, tick_clock, wait_clock):
        super()._drain_and_barrier(tick_clock, wait_clock)
        self._split_multi_waits()

    def _fresh_nop(self, engine):
        inst = self.nc.engines[engine].nop(nofuse=True).ins
        self.nc.cur_bb.bb.instructions.remove(inst)
        return inst

    def _split_multi_waits(self):
        for fn in self.nc.m.functions:
            for bb in fn.blocks:
                snapshot = list(bb.instructions)
                if not any(
                    inst.sync_info and len(inst.sync_info.on_wait) > 1
                    for inst in snapshot
                ):
                    continue
                new = []
                for inst in snapshot:
                    si = inst.sync_info
                    if si is not None and si.on_wait and len(si.on_wait) > 1:
                        waits = list(si.on_wait)
                        si.on_wait = waits[-1:]
                        inst.sync_info = si
                        for wt in waits[:-1]:
                            nop = self._fresh_nop(inst.engine)
                            nop.sync_info = mybir.SyncInfo(on_wait=[wt], on_update=[])
                            new.append(nop)
                    new.append(inst)
                bb.instructions[:] = new


def build_nc():
    nc = bass.Bass("TRN2")
    wf = nc.dram_tensor("wf", [WPC, A], mybir.dt.float32, kind="ExternalInput")
    # tau3[3*s + t] = bf16 term t of tau stripe s (exact hi/mid/lo split).
    tau3_in = nc.dram_tensor(
        "tau3", [3 * NST, CH], mybir.dt.bfloat16, kind="ExternalInput"
    )
    # mask[k, 128*s + i] = 1.0 iff row k belongs to stripe s: used as matmul
    # weights so ones.T-style contraction over all 48 rows extracts exactly
    # stripe s's three bf16 terms (their fp32 sum = tau, bit-exact).
    mask_in = nc.dram_tensor(
        "mask", [3 * NST, 128 * NST], mybir.dt.bfloat16, kind="ExternalInput"
    )
    w_in = nc.dram_tensor("W", [A], mybir.dt.float32, kind="ExternalInput")
    b_in = nc.dram_tensor("b", [1], mybir.dt.float32, kind="ExternalInput")
    out = nc.dram_tensor("out", [G, 128, F], mybir.dt.bfloat16, kind="ExternalOutput")

    f32 = mybir.dt.float32
    bf16 = mybir.dt.bfloat16

    with _TC(nc) as tc:
        with (
            tc.tile_pool(name="const", bufs=1) as const,
            tc.tile_pool(name="outs", bufs=3) as outs,
            tc.tile_pool(name="psum", bufs=2, space="PSUM") as psum,
        ):
            ones = const.tile([3, 128], bf16)
            nc.vector.memset(ones, 1.0)

            # Prefetch the Exp/Ln activation table while inputs load: the
            # first table load costs 1.28 us and must not sit on the
            # prologue's critical path.
            scr_act = const.tile([3, 128], bf16)
            nc.scalar.activation(scr_act, ones, _AF.Exp)

            # Warm the PE p-state: dummy matmuls keep the tensor engine
            # continuously busy from t~0 so the real chunk-0 matmuls run at
            # full clock instead of the 0.65 GHz cold state.
            for wi in range(NWARM):
                wpt = psum.tile([128, CH], f32, tag="pt", name=f"warm{wi}")
                nc.tensor.matmul(
                    wpt[:, 0:128], ones, ones, start=True, stop=True
                )

            # ---- input loads: wf/W/b on SP, tau3 on Pool (one 4KB/partition
            # DMA); the ACT ring never carries a DMA.
            wf_sb = const.tile([128, G, A], f32)
            nc.sync.dma_start(
                out=wf_sb, in_=wf[:].rearrange("(g p) a -> p g a", p=128)
            )
            w_ap = w_in[:]
            w_sb = const.tile([128, A], f32)
            nc.sync.dma_start(
                out=w_sb,
                in_=bass.AP(tensor=w_ap.tensor, offset=w_ap.offset, ap=[[0, 128], [1, A]]),
            )
            b_ap = b_in[:]
            b_sb = const.tile([128, 1], f32)
            nc.sync.dma_start(
                out=b_sb,
                in_=bass.AP(tensor=b_ap.tensor, offset=b_ap.offset, ap=[[0, 128], [1, 1]]),
            )
            # tau/mask head pieces cover exactly what chunk 0's first matmul
            # reads, so the PE can start ~2.5us before the full tiles land.
            tau_sb = const.tile([3 * NST, CH], bf16)
            mask_sb = const.tile([3 * NST, 128 * NST], bf16)
            nc.gpsimd.dma_start(out=tau_sb[:, :512], in_=tau3_in[:, :512])
            nc.gpsimd.dma_start(out=mask_sb[:, :128], in_=mask_in[:, :128])
            nc.gpsimd.dma_start(out=tau_sb[:, 512:], in_=tau3_in[:, 512:])
            nc.gpsimd.dma_start(out=mask_sb[:, 128:], in_=mask_in[:, 128:])

            # ---- per-worker scalars: a = ln p1 - ln p2, c = ln p2 ----
            x = const.tile([128, G], f32)
            for g in range(G):
                prod = const.tile([128, A], f32, tag=f"prod{g}")
                nc.vector.tensor_mul(prod, wf_sb[:, g, :], w_sb)
                nc.vector.reduce_sum(x[:, g : g + 1], prod, axis=mybir.AxisListType.X)

            bneg = const.tile([128, 1], f32)
            nc.vector.tensor_scalar_mul(bneg, b_sb, -1.0)
            # e = exp(-(x + b));  p1 = 1 / (1 + e)
            e = const.tile([128, G], f32)
            nc.scalar.activation(e, x, _AF.Exp, bias=bneg[:, 0:1], scale=-1.0)
            nc.vector.tensor_scalar_add(e, e, 1.0)
            p1 = const.tile([128, G], f32)
            nc.vector.reciprocal(p1, e)
            p2 = const.tile([128, G], f32)
            nc.vector.tensor_scalar(
                p2,
                p1,
                scalar1=-1.0 / (L - 1),
                scalar2=1.0 / (L - 1),
                op0=mybir.AluOpType.mult,
                op1=mybir.AluOpType.add,
            )
            lp1 = const.tile([128, G], f32)
            nc.scalar.activation(lp1, p1, _AF.Ln)
            lp2 = const.tile([128, G], f32)
            nc.scalar.activation(lp2, p2, _AF.Ln)
            a_sb = const.tile([128, G], f32)
            nc.vector.tensor_sub(a_sb, lp1, lp2)

            # ---- main loop: PE-broadcast tau piece -> ACT exp -> DMA out ----
            dma_engines = [nc.sync, nc.gpsimd]
            qi = 0
            for pi, (s, f0, sz) in enumerate(PIECES):
                pt = psum.tile([128, CH], f32, tag="pt", name=f"pt{pi}")
                # The PE sleeps between pieces (PSUM WAR) and restarts in the
                # cold p-state; a tiny leading matmul absorbs that penalty so
                # the 512-row ones run at the mid clock.
                spans = [(0, 64), (64, 448)] if sz >= 512 else [(0, sz)]
                spans += [(n, 512) for n in range(512, sz, 512)]
                for n, w in spans:
                    nc.tensor.matmul(
                        pt[:, f0 + n : f0 + n + w],
                        mask_sb[:, 128 * s : 128 * (s + 1)],
                        tau_sb[:, f0 + n : f0 + n + w],
                        start=True,
                        stop=True,
                    )
                base = s * CH + f0
                for g in range(G):
                    ot = outs.tile(
                        [128, CH], bf16, tag=f"ot{g}", name=f"ot{g}_p{pi}", bufs=4
                    )
                    nc.scalar.activation(
                        ot[:, :sz],
                        pt[:, f0 : f0 + sz],
                        _AF.Exp,
                        bias=lp2[:, g : g + 1],
                        scale=a_sb[:, g : g + 1],
                    )
                    dma_engines[qi % 2].dma_start(
                        out=out[g, :, base : base + sz], in_=ot[:, :sz]
                    )
                    qi += 1
    return nc


def _split3_bf16(x32):
    """Exact 3-term bf16 decomposition of fp32 (hi+mid+lo == x bit-exact)."""
    import ml_dtypes

    bf = ml_dtypes.bfloat16
    hi = x32.astype(bf)
    r1 = x32 - hi.astype(np.float32)
    mid = r1.astype(bf)
    r2 = r1 - mid.astype(np.float32)
    lo = r2.astype(bf)
    return np.stack([hi, mid, lo], axis=0)


def _pack_tau3(tau_flat):
    """[3, F] split -> [48, 2048]: row 3*s + t = term t of stripe s."""
    t3 = _split3_bf16(tau_flat)  # [3, F]
    return np.ascontiguousarray(
        t3.reshape(3, NST, CH).transpose(1, 0, 2).reshape(3 * NST, CH)
    )


def _stripe_mask():
    """[48, 16*128] bf16: mask[k, 128*s + i] = 1.0 iff k // 3 == s."""
    import ml_dtypes

    m = np.zeros((3 * NST, NST, 128), dtype=ml_dtypes.bfloat16)
    for s in range(NST):
        m[3 * s : 3 * (s + 1), s, :] = 1.0
    return np.ascontiguousarray(m.reshape(3 * NST, NST * 128))


_NC = None


def kernel(inputs, W, b, worker_num=WN, task_num=TN, edge_type=L, ability_num=A, **_kw):
    global _NC
    inputs = np.ascontiguousarray(np.asarray(inputs, dtype=np.float32))
    W = np.asarray(W, dtype=np.float32).reshape(A)
    b = np.asarray(b, dtype=np.float32).reshape(1)
    assert inputs.shape == (WN + TN, A)

    wf = inputs[:WN, :A]
    tau3 = _pack_tau3(inputs[WN:, :L].reshape(F))

    if _NC is None:
        _NC = build_nc()

    mask = _stripe_mask()
    in_maps = [
        {
            "wf": np.ascontiguousarray(wf[k * WPC : (k + 1) * WPC]),
            "tau3": tau3,
            "mask": mask,
            "W": W,
            "b": b,
        }
        for k in range(NCORES)
    ]
    res = run_bass_kernel_spmd(_NC, in_maps, core_ids=list(range(NCORES)))
    parts = [
        r["out"].astype(np.float32).reshape(WPC, TN, L) for r in res.results
    ]
    return np.concatenate(parts, axis=0)


# revision 16
# speedup vs baseline: 1.9537x; 1.0510x over previous
"""Trainium2 Bass kernel for nn_Decoder (worker/task label-probability decoder).

Math:
    worker_feature = inputs[:2048, :64]          # [Wn, A]
    tau            = inputs[2048:, :16]          # [T, L]
    p1 = sigmoid(worker_feature @ W + b)         # [Wn, 1]
    p2 = (1 - p1) / (L - 1)
    P[i, j, l] = p1[i]^tau[j,l] * p2[i]^(1 - tau[j,l])
               = exp(a[i] * tau[j,l] + c[i]),  a = ln p1 - ln p2, c = ln p2

Sharding: pure data parallel over the worker axis (dim 0), 256 workers per
core across 8 cores; tau/W/b replicated. No communication.

Per-core layout: workers on SBUF partitions (2 groups of 128), task axis
flattened on the free dimension. tau ships as an exact 3-term bf16 split,
striped [48 x 2048] so the load is one cheap DMA; the otherwise-idle tensor
engine replicates each stripe to all 128 partitions (ones.T @ tau3 sums the
three bf16 terms in fp32 -> exact tau in PSUM). The scalar engine computes
Exp(a*tau + c) with per-partition scale/bias, writing bf16 tiles (the 2e-2
rel-err budget dwarfs bf16's 2e-3 rounding), halving HBM write traffic.

The scalar engine is the roofline (~61 us of Exp); everything else is
arranged to keep it saturated: output DMAs go only on the SP and Pool
queues, the Exp activation table is prefetched via a dummy op during the
input loads, dummy matmuls warm the PE p-state so chunk 0 isn't slowed by
the cold tensor engine, and the first/last chunks are subdivided to shrink
pipeline fill/drain.
"""

import numpy as np

try:
    import concourse.bass as bass  # noqa: F401
except ImportError:  # fall back to the container's repo checkout
    import sys

    for _p in ("/root/.axon_site/_ro/trn_rl_repo", "/opt/trn_rl_repo"):
        if _p not in sys.path:
            sys.path.append(_p)

import concourse.bass as bass
import concourse.tile as tile
from concourse import library_config, mybir
from concourse.bass_utils import run_bass_kernel_spmd
from concourse.vector_clock import ScopedClock

WN = 2048  # workers total
TN = 2048  # tasks
L = 16  # edge types / labels
A = 64  # ability features
NCORES = 8
WPC = WN // NCORES  # workers per core (256)
G = WPC // 128  # partition groups per core (2)
F = TN * L  # flattened task axis (32768)
CH = 2048  # stripe length: PSUM ping-pong granularity + per-chunk output DMA
NST = F // CH  # tau stripes (16)
NWARM = 24  # dummy matmuls to warm the PE p-state

SW = 8192  # SBUF stage width: 4 PSUM pieces aggregated per stage
NSTG = F // SW  # stages (4)

_AF = mybir.ActivationFunctionType


class _TC(tile.TileContext):
    """TileContext legalized for a walrus that allows one sync-wait per inst.

    The walrus build in this container rejects any instruction carrying more
    than one sync-wait command. After Tile's normal scheduling + the exit
    drain/barrier, rewrite every multi-wait instruction into a chain of
    same-engine NOPs (one wait each) followed by the instruction with the
    final wait.
    """

    def _drain_and_barrier(self, tick_clock, wait_clock):
        super()._drain_and_barrier(tick_clock, wait_clock)
        self._split_multi_waits()

    def _fresh_nop(self, engine):
        inst = self.nc.engines[engine].nop(nofuse=True).ins
        self.nc.cur_bb.bb.instructions.remove(inst)
        return inst

    def _split_multi_waits(self):
        for fn in self.nc.m.functions:
            for bb in fn.blocks:
                snapshot = list(bb.instructions)
                if not any(
                    inst.sync_info and len(inst.sync_info.on_wait) > 1
                    for inst in snapshot
                ):
                    continue
                new = []
                for inst in snapshot:
                    si = inst.sync_info
                    if si is not None and si.on_wait and len(si.on_wait) > 1:
                        waits = list(si.on_wait)
                        si.on_wait = waits[-1:]
                        inst.sync_info = si
                        for wt in waits[:-1]:
                            nop = self._fresh_nop(inst.engine)
                            nop.sync_info = mybir.SyncInfo(on_wait=[wt], on_update=[])
                            new.append(nop)
                    new.append(inst)
                bb.instructions[:] = new


def build_nc():
    nc = bass.Bass("TRN2")
    wf = nc.dram_tensor("wf", [WPC, A], mybir.dt.float32, kind="ExternalInput")
    # tau3[3*s + t] = bf16 term t of tau stripe s (exact hi/mid/lo split).
    tau3_in = nc.dram_tensor(
        "tau3", [3 * NST, CH], mybir.dt.bfloat16, kind="ExternalInput"
    )
    # mask[k, 128*s + i] = 1.0 iff row k belongs to stripe s: used as matmul
    # weights so ones.T-style contraction over all 48 rows extracts exactly
    # stripe s's three bf16 terms (their fp32 sum = tau, bit-exact).
    mask_in = nc.dram_tensor(
        "mask", [3 * NST, 128 * NST], mybir.dt.bfloat16, kind="ExternalInput"
    )
    w_in = nc.dram_tensor("W", [A], mybir.dt.float32, kind="ExternalInput")
    b_in = nc.dram_tensor("b", [1], mybir.dt.float32, kind="ExternalInput")
    out = nc.dram_tensor("out", [G, 128, F], mybir.dt.bfloat16, kind="ExternalOutput")

    f32 = mybir.dt.float32
    bf16 = mybir.dt.bfloat16

    with _TC(nc) as tc:
        with (
            tc.tile_pool(name="const", bufs=1) as const,
            tc.tile_pool(name="outs", bufs=3) as outs,
            tc.tile_pool(name="psum", bufs=2, space="PSUM") as psum,
        ):
            ones = const.tile([3, 128], bf16)
            nc.vector.memset(ones, 1.0)

            # Prefetch the Exp/Ln activation table while inputs load: the
            # first table load costs 1.28 us and must not sit on the
            # prologue's critical path.
            scr_act = const.tile([3, 128], bf16)
            nc.scalar.activation(scr_act, ones, _AF.Exp)

            # Warm the PE p-state: dummy matmuls keep the tensor engine
            # continuously busy from t~0 so the real chunk-0 matmuls run at
            # full clock instead of the 0.65 GHz cold state.
            for wi in range(NWARM):
                wpt = psum.tile([128, CH], f32, tag="pt", name=f"warm{wi}")
                nc.tensor.matmul(
                    wpt[:, 0:128], ones, ones, start=True, stop=True
                )

            # ---- input loads: wf/W/b on SP, tau3 on Pool (one 4KB/partition
            # DMA); the ACT ring never carries a DMA.
            wf_sb = const.tile([128, G, A], f32)
            nc.sync.dma_start(
                out=wf_sb, in_=wf[:].rearrange("(g p) a -> p g a", p=128)
            )
            w_ap = w_in[:]
            w_sb = const.tile([128, A], f32)
            nc.sync.dma_start(
                out=w_sb,
                in_=bass.AP(tensor=w_ap.tensor, offset=w_ap.offset, ap=[[0, 128], [1, A]]),
            )
            b_ap = b_in[:]
            b_sb = const.tile([128, 1], f32)
            nc.sync.dma_start(
                out=b_sb,
                in_=bass.AP(tensor=b_ap.tensor, offset=b_ap.offset, ap=[[0, 128], [1, 1]]),
            )
            # tau/mask head pieces cover exactly what chunk 0's first matmul
            # reads, so the PE can start ~2.5us before the full tiles land.
            tau_sb = const.tile([3 * NST, CH], bf16)
            mask_sb = const.tile([3 * NST, 128 * NST], bf16)
            nc.gpsimd.dma_start(out=tau_sb[:, :512], in_=tau3_in[:, :512])
            nc.gpsimd.dma_start(out=mask_sb[:, :128], in_=mask_in[:, :128])
            nc.gpsimd.dma_start(out=tau_sb[:, 512:], in_=tau3_in[:, 512:])
            nc.gpsimd.dma_start(out=mask_sb[:, 128:], in_=mask_in[:, 128:])

            # ---- per-worker scalars: a = ln p1 - ln p2, c = ln p2 ----
            x = const.tile([128, G], f32)
            for g in range(G):
                prod = const.tile([128, A], f32, tag=f"prod{g}")
                nc.vector.tensor_mul(prod, wf_sb[:, g, :], w_sb)
                nc.vector.reduce_sum(x[:, g : g + 1], prod, axis=mybir.AxisListType.X)

            bneg = const.tile([128, 1], f32)
            nc.vector.tensor_scalar_mul(bneg, b_sb, -1.0)
            # e = exp(-(x + b));  p1 = 1 / (1 + e)
            e = const.tile([128, G], f32)
            nc.scalar.activation(e, x, _AF.Exp, bias=bneg[:, 0:1], scale=-1.0)
            nc.vector.tensor_scalar_add(e, e, 1.0)
            p1 = const.tile([128, G], f32)
            nc.vector.reciprocal(p1, e)
            p2 = const.tile([128, G], f32)
            nc.vector.tensor_scalar(
                p2,
                p1,
                scalar1=-1.0 / (L - 1),
                scalar2=1.0 / (L - 1),
                op0=mybir.AluOpType.mult,
                op1=mybir.AluOpType.add,
            )
            lp1 = const.tile([128, G], f32)
            nc.scalar.activation(lp1, p1, _AF.Ln)
            lp2 = const.tile([128, G], f32)
            nc.scalar.activation(lp2, p2, _AF.Ln)
            a_sb = const.tile([128, G], f32)
            nc.vector.tensor_sub(a_sb, lp1, lp2)

            # ---- main loop ----
            # PE broadcasts tau stripe s into a PSUM piece; the otherwise-idle
            # DVE drains each piece to a [128, 8192] f32 SBUF stage (freeing
            # the 2-deep PSUM ring early, so the PE never gates the scalar
            # engine); ACT then runs big 8192-wide Exp ops out of SBUF (fewer
            # per-instruction overheads) and the bf16 result streams to HBM
            # in 2048-chunks alternating between the SP and Pool queues.
            dma_engines = [nc.sync, nc.gpsimd]
            qi = 0

            def act_reads(i, g):
                # First stage, first group chases the copy pipeline in small
                # reads; the very last ACT is small so the drain tail is short.
                if i == 0 and g == 0:
                    return [(0, 512), (512, 1536), (2048, 2048), (4096, 2048), (6144, 2048)]
                if i == NSTG - 1 and g == 1:
                    return [(0, 2048), (2048, 2048), (4096, 2048), (6144, 1536), (7680, 512)]
                return [(0, SW)]

            stg = None
            for p in range(NST):
                pt = psum.tile([128, CH], f32, tag="pt", name=f"pt{p}")
                for n in range(0, CH, 512):
                    nc.tensor.matmul(
                        pt[:, n : n + 512],
                        mask_sb[:, 128 * p : 128 * (p + 1)],
                        tau_sb[:, n : n + 512],
                        start=True,
                        stop=True,
                    )
                i, q = divmod(p, 4)
                if q == 0:
                    stg = outs.tile([128, SW], f32, tag="stg", name=f"stg{i}", bufs=2)
                # piece 0 copied in two halves so the first ACT fires ASAP
                spans = [(0, 512), (512, 1536)] if p == 0 else [(0, CH)]
                for c0, cw in spans:
                    nc.vector.tensor_scalar_mul(
                        stg[:, q * CH + c0 : q * CH + c0 + cw],
                        pt[:, c0 : c0 + cw],
                        1.0,
                    )
                if q != 3:
                    continue
                for g in range(G):
                    ot = outs.tile(
                        [128, SW], bf16, tag=f"ot{g}", name=f"ot{g}_s{i}", bufs=2
                    )
                    for off, w in act_reads(i, g):
                        nc.scalar.activation(
                            ot[:, off : off + w],
                            stg[:, off : off + w],
                            _AF.Exp,
                            bias=lp2[:, g : g + 1],
                            scale=a_sb[:, g : g + 1],
                        )
                        for d0 in range(off, off + w, CH):
                            dw = min(CH, off + w - d0)
                            dma_engines[qi % 2].dma_start(
                                out=out[g, :, i * SW + d0 : i * SW + d0 + dw],
                                in_=ot[:, d0 : d0 + dw],
                            )
                            qi += 1
    return nc


def _split3_bf16(x32):
    """Exact 3-term bf16 decomposition of fp32 (hi+mid+lo == x bit-exact)."""
    import ml_dtypes

    bf = ml_dtypes.bfloat16
    hi = x32.astype(bf)
    r1 = x32 - hi.astype(np.float32)
    mid = r1.astype(bf)
    r2 = r1 - mid.astype(np.float32)
    lo = r2.astype(bf)
    return np.stack([hi, mid, lo], axis=0)


def _pack_tau3(tau_flat):
    """[3, F] split -> [48, 2048]: row 3*s + t = term t of stripe s."""
    t3 = _split3_bf16(tau_flat)  # [3, F]
    return np.ascontiguousarray(
        t3.reshape(3, NST, CH).transpose(1, 0, 2).reshape(3 * NST, CH)
    )


def _stripe_mask():
    """[48, 16*128] bf16: mask[k, 128*s + i] = 1.0 iff k // 3 == s."""
    import ml_dtypes

    m = np.zeros((3 * NST, NST, 128), dtype=ml_dtypes.bfloat16)
    for s in range(NST):
        m[3 * s : 3 * (s + 1), s, :] = 1.0
    return np.ascontiguousarray(m.reshape(3 * NST, NST * 128))


_NC = None


def kernel(inputs, W, b, worker_num=WN, task_num=TN, edge_type=L, ability_num=A, **_kw):
    global _NC
    inputs = np.ascontiguousarray(np.asarray(inputs, dtype=np.float32))
    W = np.asarray(W, dtype=np.float32).reshape(A)
    b = np.asarray(b, dtype=np.float32).reshape(1)
    assert inputs.shape == (WN + TN, A)

    wf = inputs[:WN, :A]
    tau3 = _pack_tau3(inputs[WN:, :L].reshape(F))

    if _NC is None:
        _NC = build_nc()

    mask = _stripe_mask()
    in_maps = [
        {
            "wf": np.ascontiguousarray(wf[k * WPC : (k + 1) * WPC]),
            "tau3": tau3,
            "mask": mask,
            "W": W,
            "b": b,
        }
        for k in range(NCORES)
    ]
    res = run_bass_kernel_spmd(_NC, in_maps, core_ids=list(range(NCORES)))
    parts = [
        r["out"].astype(np.float32).reshape(WPC, TN, L) for r in res.results
    ]
    return np.concatenate(parts, axis=0)
